# revision 26
# baseline (speedup 1.0000x reference)
"""KANLinear (RBF-KAN) Trainium2 kernel.

Math (matches the reference):
  x_flat [B=8192, IN=1024]
  base   = silu(x) @ (base_w.T) + base_b
  basis[b,i,g] = exp(-(d*(x[b,i]-grid[g]))**2),  grid = linspace(-2,2,8), d = 1/(delta+1e-6)
  spline = einsum('big,oig->bo', basis, spline_w)
  out    = base + spline        [B, OUT=1024]

Implementation:
  - Data parallel over tokens: 8 cores x 1024 tokens each; weights replicated.
  - The spline contraction is a [tok, IN*G=8192] @ [8192, OUT] matmul. Per core we
    hold spline_w (transposed to [G*IN, OUT], bf16, 16MB) resident in SBUF and run
    bf16 matmuls with K accumulated in PSUM (fp32).
  - Basis tiles are produced on the fly:
      v = (x - 2g)*x              (one VectorE scalar_tensor_tensor, fp32)
      basis = Exp(-d^2*v)         (one ScalarE activation, bf16 out)
    and the remaining constant factor exp(-d^2*g^2) is folded into the spline
    weights host-side, so exp needs no per-grid bias const (no const-AP setup,
    no extra engine barrier at the front).
  - silu(x) is computed as x*(1+tanh(x/2)) (tanh lives in the same ACT table set
    as exp, avoiding table switches); the 0.5 factor is folded into base_w host-side.
  - base_b is zero for this problem's generator; when it is zero the bias path is
    compiled out entirely (a rank-1 ones x bias matmul fallback exists otherwise).
  - Layout: out[tokens(part), out(free)] bf16 so the result DMAs out contiguously.
"""

import os
import sys

os.environ.setdefault("MYCRO_LOCAL_CACHE", "1")
for _p in ("/opt/trn_rl_repo", "/root/.axon_site/_ro/trn_rl_repo"):
    if os.path.isdir(_p) and _p not in sys.path:
        sys.path.insert(0, _p)

import numpy as np
import ml_dtypes

IN_F = 1024
OUT_F = 1024
G = 8
GRID_LO, GRID_HI = -2.0, 2.0
NCORES = 8
TOK = 8192
TCORE = TOK // NCORES   # 1024 tokens per core
NG = 2                  # token groups per core
GTOK = TCORE // NG      # 512 tokens per group
MT = GTOK // 128        # 4 psum m-tiles (128 tokens) per group
KS = G * (IN_F // 128)  # 64 spline k-tiles
KB = IN_F // 128        # 8 base k-tiles

_DELTA = float((GRID_HI - GRID_LO) / (G - 1))
_D = 1.0 / (_DELTA + 1e-6)
# match jax's f32 linspace values
_GRID = np.linspace(GRID_LO, GRID_HI, G, dtype=np.float32).astype(np.float64)

TRACE = False
LAST_RESULT = None
_NC_CACHE = {}


def build_nc(with_bias=False):
    from concourse import bacc
    import concourse.mybir as mybir
    import concourse.tile as tile

    F32 = mybir.dt.float32
    BF16 = mybir.dt.bfloat16
    Alu = mybir.AluOpType
    Act = mybir.ActivationFunctionType

    nc = bacc.Bacc("TRN2", target_bir_lowering=False)
    xg_d = nc.dram_tensor("xg", [NG, 128, KB, GTOK], F32, kind="ExternalInput")
    spl_d = nc.dram_tensor("spline", [KS * 128, OUT_F], BF16, kind="ExternalInput")
    bw_d = nc.dram_tensor("basew", [IN_F, OUT_F], BF16, kind="ExternalInput")
    if with_bias:
        bb_d = nc.dram_tensor("brow", [1, OUT_F], BF16, kind="ExternalInput")
    out_d = nc.dram_tensor("out", [TCORE, OUT_F], BF16, kind="ExternalOutput")

    d2 = _D * _D

    with tile.TileContext(nc) as tc:
        with (
            tc.tile_pool(name="const", bufs=1) as cpool,
            tc.tile_pool(name="xg", bufs=2) as xpool,
            tc.tile_pool(name="silu", bufs=1) as spool,
            tc.tile_pool(name="tanh", bufs=1) as tpool,
            tc.tile_pool(name="v", bufs=2) as vpool,
            tc.tile_pool(name="basis", bufs=3) as bpool,
            tc.tile_pool(name="osb", bufs=3) as opool,
            tc.tile_pool(name="psum", bufs=8, space="PSUM") as ppool,
        ):
            spl_sb = cpool.tile([128, KS, OUT_F], BF16)
            bw_sb = cpool.tile([128, KB, OUT_F], BF16)
            wup_sb = cpool.tile([128, 128], BF16)
            if with_bias:
                ones_sb = cpool.tile([1, 128], BF16)
                brow_sb = cpool.tile([1, OUT_F], BF16)
            spl_view = spl_d[:].rearrange("(k p) n -> p k n", p=128)
            bw_view = bw_d[:].rearrange("(k p) n -> p k n", p=128)

            for grp in range(NG):
                xg = xpool.tile([128, KB, GTOK], F32, tag="xg", name=f"xg_g{grp}")
                ps = [
                    [
                        ppool.tile([128, 512], F32, tag="ps",
                                   name=f"ps_g{grp}m{m}n{n}")
                        for n in range(2)
                    ]
                    for m in range(MT)
                ]
                if grp == 0:
                    # minimal-latency head. K-tiles run i-major (k = i*G + g),
                    # so the first 8 k-tiles all use x i-block 0 and spline
                    # columns 0..7 — one small x piece plus a lean column
                    # stream feeds the whole head.
                    nc.sync.dma_start(xg[:, 0:1, :], xg_d[grp, :, 0:1, :])
                    nc.sync.dma_start(spl_sb[:, 0:1, :], spl_view[:, 0:1, :])
                    nc.sync.dma_start(spl_sb[:, 1:2, :], spl_view[:, 1:2, :])
                    nc.sync.dma_start(spl_sb[:, 2:4, :], spl_view[:, 2:4, :])
                    nc.sync.dma_start(xg[:, 1:2, :], xg_d[grp, :, 1:2, :])
                    nc.sync.dma_start(spl_sb[:, 4:8, :], spl_view[:, 4:8, :])
                    nc.sync.dma_start(xg[:, 2:4, :], xg_d[grp, :, 2:4, :])
                    nc.sync.dma_start(spl_sb[:, 8:16, :], spl_view[:, 8:16, :])
                    nc.sync.dma_start(xg[:, 4:8, :], xg_d[grp, :, 4:8, :])
                    for c0, c1 in ((16, 24), (24, 32), (32, 40), (40, 48),
                                   (48, 56), (56, 64)):
                        nc.sync.dma_start(
                            spl_sb[:, c0:c1, :],
                            spl_view[:, c0:c1, :],
                        )
                    nc.sync.dma_start(bw_sb[:], bw_view[:])
                    if with_bias:
                        nc.vector.memset(ones_sb[:], 1.0)
                        nc.sync.dma_start(brow_sb[:], bb_d[:])
                    # HAM warmup: K=128 dummy matmuls (K=1 ones don't register
                    # as PE activity) to push the activity monitor to 8/8
                    # while the first DMAs land. Writes land in ps[...,0:128]
                    # and are discarded by the start=True of the first real
                    # matmul on each m-tile.
                    nc.vector.memset(wup_sb[:], 0.0)
                    for w in range(24):
                        nc.tensor.matmul(
                            ps[w % MT][0][:, 0:128], wup_sb[:], wup_sb[:],
                            start=True, stop=True,
                        )
                else:
                    nc.sync.dma_start(xg[:], xg_d[grp, :, :, :])
                silu = spool.tile([128, KB, GTOK], BF16)

                for k in range(KS):
                    i, g = divmod(k, G)
                    gval = float(_GRID[g])
                    v = vpool.tile([128, GTOK], F32)
                    basis = bpool.tile([128, GTOK], BF16)
                    if grp == 0 and k == 0:
                        # pipeline the very first basis tile in halves so the
                        # first matmuls issue as soon as 256 tokens of x land
                        halves = [(0, 256), (256, 512)]
                    else:
                        halves = [(0, 512)]
                    for lo, hi in halves:
                        nc.vector.scalar_tensor_tensor(
                            v[:, lo:hi], xg[:, i, lo:hi], -2.0 * gval,
                            xg[:, i, lo:hi],
                            op0=Alu.add, op1=Alu.mult,
                        )
                        nc.scalar.activation(
                            basis[:, lo:hi], v[:, lo:hi], Act.Exp,
                            scale=float(-d2),
                        )
                        for m in range(lo // 128, hi // 128):
                            lhsT = basis[:, m * 128:(m + 1) * 128]
                            for n in range(2):
                                nc.tensor.matmul(
                                    ps[m][n][:],
                                    lhsT,
                                    spl_sb[:, k, n * 512:(n + 1) * 512],
                                    start=(k == 0), stop=False,
                                )
                    # silu2 = x*(1+tanh(x/2)) = 2*silu(x); 0.5 folded into
                    # basew. Half a tile per slot (two slots per i-block,
                    # every ~3rd k) so ACT/DVE absorb the extra op without
                    # ever stalling the exp stream.
                    sl2 = None
                    if 10 <= k <= 52 and k % 6 == 4:
                        i2, sl2 = (k - 10) // 6, slice(0, 256)
                    elif 13 <= k <= 55 and k % 6 == 1:
                        i2, sl2 = (k - 13) // 6, slice(256, 512)
                    if sl2 is not None:
                        t = tpool.tile([128, GTOK], F32)
                        nc.scalar.activation(
                            t[:, sl2], xg[:, i2, sl2], Act.Tanh, scale=0.5)
                        nc.vector.scalar_tensor_tensor(
                            silu[:, i2, sl2], t[:, sl2], 1.0, xg[:, i2, sl2],
                            op0=Alu.add, op1=Alu.mult,
                        )

                def base_mms(m, kb):
                    last = not with_bias and kb == KB - 1
                    lhsT = silu[:, kb, m * 128:(m + 1) * 128]
                    for n in range(2):
                        nc.tensor.matmul(
                            ps[m][n][:],
                            lhsT,
                            bw_sb[:, kb, n * 512:(n + 1) * 512],
                            start=False, stop=last,
                        )

                def bias_mms(m):
                    for n in range(2):
                        nc.tensor.matmul(
                            ps[m][n][:],
                            ones_sb[0:1, :],
                            brow_sb[0:1, n * 512:(n + 1) * 512],
                            start=False, stop=True,
                        )

                # base phase m-major: each m-tile finishes its base matmuls
                # then evicts inline, so evictions overlap the next m-tile's
                # matmuls (or the next group's spline) and the psum buffers
                # free in the order the next group reuses them.
                last_grp = grp == NG - 1
                for m in range(MT):
                    mg = grp * MT + m
                    o = opool.tile([128, OUT_F], BF16, tag="osb", name=f"o_{mg}")
                    if last_grp and m == MT - 1 and not with_bias:
                        # final m-tile: n-major so the first half's eviction
                        # copy overlaps the second half's matmuls, and the
                        # last copy splits across DVE+ACT
                        for n in range(2):
                            for kb in range(KB):
                                nc.tensor.matmul(
                                    ps[m][n][:],
                                    silu[:, kb, m * 128:(m + 1) * 128],
                                    bw_sb[:, kb, n * 512:(n + 1) * 512],
                                    start=False, stop=kb == KB - 1,
                                )
                            if n == 0:
                                nc.vector.tensor_copy(o[:, 0:512], ps[m][0][:])
                            else:
                                nc.vector.tensor_copy(
                                    o[:, 512:768], ps[m][1][:, 0:256])
                                nc.scalar.copy(
                                    o[:, 768:1024], ps[m][1][:, 256:512])
                    else:
                        for kb in range(KB):
                            base_mms(m, kb)
                        if with_bias:
                            bias_mms(m)
                        nc.vector.tensor_copy(o[:, 0:512], ps[m][0][:])
                        nc.scalar.copy(o[:, 512:1024], ps[m][1][:])
                    nc.sync.dma_start(out_d[mg * 128:(mg + 1) * 128, :], o[:])

    nc.compile()
    return nc


def _host_prep(x, base_w, base_b, spline_w):
    x = np.asarray(x, dtype=np.float32)
    base_w = np.asarray(base_w, dtype=np.float32)
    base_b = np.asarray(base_b, dtype=np.float32)
    spline_w = np.asarray(spline_w, dtype=np.float32)

    x_flat = np.ascontiguousarray(x.reshape(TOK, IN_F))
    # [OUT, IN, G] -> [G, IN, OUT], then order k-tiles i-major (k = i*G + g)
    # to match the kernel's K loop. Fold exp(-d^2*g^2) (the constant part of
    # the RBF exponent) into the weights so the device-side exp needs no
    # per-grid bias.
    d2 = _D * _D
    gscale = np.exp(-d2 * _GRID * _GRID)  # [G], float64
    spl = spline_w.astype(np.float64).transpose(2, 1, 0) * gscale[:, None, None]
    spl = spl.reshape(G, KB, 128, OUT_F).transpose(1, 0, 2, 3)
    spl = np.ascontiguousarray(spl.reshape(G * IN_F, OUT_F)).astype(ml_dtypes.bfloat16)
    bw = np.ascontiguousarray(0.5 * base_w.T).astype(ml_dtypes.bfloat16)
    with_bias = bool(np.any(base_b != 0.0))
    brow = np.ascontiguousarray(base_b.reshape(1, OUT_F)).astype(ml_dtypes.bfloat16)

    in_maps = []
    for c in range(NCORES):
        shard = x_flat[c * TCORE:(c + 1) * TCORE, :]   # [tok, in]
        xT = shard.T                                    # [in, tok]
        # [in, tok] -> [i, p, grp, t] -> [grp, p, i, t]
        xg = np.ascontiguousarray(
            xT.reshape(KB, 128, NG, GTOK).transpose(2, 1, 0, 3)
        )
        m = {"xg": xg, "spline": spl, "basew": bw}
        if with_bias:
            m["brow"] = brow
        in_maps.append(m)
    return in_maps, with_bias


def kernel(x, base_w, base_b, spline_w):
    global LAST_RESULT
    from concourse.bass_utils import run_bass_kernel_spmd

    in_maps, with_bias = _host_prep(x, base_w, base_b, spline_w)
    if with_bias not in _NC_CACHE:
        _NC_CACHE[with_bias] = build_nc(with_bias=with_bias)
    res = run_bass_kernel_spmd(
        _NC_CACHE[with_bias], in_maps, core_ids=list(range(NCORES)), trace=TRACE
    )
    LAST_RESULT = res
    outs = [np.asarray(r["out"]).astype(np.float32) for r in res.results]
    full = np.concatenate(outs, axis=0)  # [8192, 1024]
    return full.reshape(4, 2048, OUT_F)
